# revision 31
# baseline (speedup 1.0000x reference)
"""GCN graph-classification kernel for 8 Trainium2 NeuronCores.

Model (PyG-style GCNConv x2 + mean pool + log_softmax):
    h   = x @ W1
    H1  = relu(Ahat @ h + b1)          Ahat = D^-1/2 (A + I) D^-1/2
    H2  = Ahat @ (H1 @ W2) + b2
    out = log_softmax(mean-pool-per-graph(H2))

Distribution strategy (8 cores):
  * nodes partitioned contiguously (6250/core); per-core in-degree-sorted
    permutation so destination tiles have homogeneous degrees.
  * layer 1: h computed locally (bf16), dis-prescaled, AllGathered in TWO
    pipelined Shared-output collectives (tiles 0-24 -> table A of 25600
    rows, tiles 25-48 -> table B of 24576 rows; both fit the int16 gather
    index range, so no lo/hi split is needed and the A-gathers start as
    soon as AG1 lands, while the second half of phase B still runs).
    Each core aggregates its own nodes' in-edges with dma_gather (256B
    bf16 edge messages) + one-hot selector matmuls accumulating in PSUM.
    Gathers are spread round-robin across 4 SWDGE queues so descriptor
    generation runs on 4 Q7 core-pairs in parallel. Self-loops are folded
    into one identity-matmul per tile from the locally kept h tiles.
  * layer 2 + pooling folded:  pooled = (Q @ H1) @ W2 + b2  with
    Q = P_mean @ Ahat  (500 x 50000, built dense-per-node-tile on host).
    Each core contracts its own H1 tiles against its Q blocks -> partial
    per-graph sums -> AllReduce (500x128 floats) -> W2 -> log_softmax.
  All symmetric-norm factors, mean-pool counts and the permutation are
  folded into host-built index/selector/Q arrays (pure index-side prep).
"""

import os
import numpy as np

import concourse.bacc as bacc
import concourse.mybir as mybir
from concourse import tile
from concourse.bass_utils import run_bass_kernel_spmd

# ---------------------------------------------------------------- constants
N, E, F, HID, C, G = 50000, 600000, 128, 128, 16, 500
P = 8                      # NeuronCores
NV = N // P                # nodes per core
NT = (NV + 127) // 128     # node tiles per core (49)
TPAD = NT * 128            # padded per-core node count (6272)
GP = 512                   # padded graph count
GT = GP // 128             # graph tiles
HALF = N // 2              # gather-table half (int16 index limit):
                           # lo = cores 0-3's rows, hi = cores 4-7's
XSPL = 25 * 128            # xT load split point (two parallel DMAs)
NB = 14                    # layer-1 gather batches
NQ = 4                     # SWDGE queues (parallel gather descriptor gen)

AF = mybir.ActivationFunctionType
ALU = mybir.AluOpType

LAST_EXEC_NS = None
LAST_RESULT = None


def _install_profile_hook():
    """The agent image's antenv lacks axon_hooks; shim it so
    run_bass_kernel_spmd(trace=True) can capture NTFF profiles."""
    import sys
    import types
    if "antenv.axon_hooks" in sys.modules:
        return True
    try:
        from trn_agent_boot.trn_boot import _ntff_profile_via_ctypes
        hook = _ntff_profile_via_ctypes("/opt/axon/libaxon_pjrt.so")
        if hook is None:
            return False
        mod = types.ModuleType("antenv.axon_hooks")
        mod._hook = hook
        mod.get_axon_ntff_profile_hook = lambda: mod._hook

        def _set(h):
            mod._hook = h
        mod.set_axon_ntff_profile_hook = _set
        sys.modules["antenv.axon_hooks"] = mod
        import antenv
        antenv.axon_hooks = mod
        return True
    except Exception as e:  # profiling is best-effort
        print(f"profile hook unavailable: {e}")
        return False


# ---------------------------------------------------------------- host prep
def _preprocess(x, W1, b1, W2, b2, edge_src, edge_dst, batch):
    import ml_dtypes
    f32 = np.float32
    bf16 = ml_dtypes.bfloat16
    src = np.asarray(edge_src, np.int64)
    dst = np.asarray(edge_dst, np.int64)
    bat = np.asarray(batch, np.int64)
    x = np.asarray(x, f32)

    deg = np.bincount(dst, minlength=N).astype(np.float64) + 1.0
    dis = 1.0 / np.sqrt(deg)
    cnt = np.maximum(np.bincount(bat, minlength=G), 1).astype(np.float64)

    # per-core degree-descending node permutation
    pos = np.empty(N, np.int64)
    order = np.empty(N, np.int64)      # order[k*NV+j] = node at position j
    for k in range(P):
        v0 = k * NV
        loc = np.argsort(-deg[v0:v0 + NV], kind="stable")
        order[v0:v0 + NV] = v0 + loc
        pos[v0 + loc] = np.arange(NV)

    # ---- layer-1 gather edges (no self-loops; those come from local h
    # tiles via an identity matmul), grouped (core, tile, src-table)
    d_own = dst // NV
    d_pos = pos[dst]
    t_of = d_pos // 128
    dloc_v = (d_pos % 128).astype(f32)
    slot = (src // NV) * NV + pos[src]   # row of src in AllGathered h
    is_B = (slot >= HALF).astype(np.int64)
    idx_v = (slot - is_B * HALF).astype(np.int16)

    key = (d_own * NT + t_of) * 2 + is_B
    ordr = np.argsort(key, kind="stable")
    idx_s = idx_v[ordr]
    dloc_s = dloc_v[ordr]
    bounds = np.searchsorted(key[ordr], np.arange(P * NT * 2 + 1))
    cnts = np.diff(bounds).reshape(P, NT, 2)
    CH = -(-cnts // 128)               # chunks per (core, tile, table)
    CH = CH.max(axis=0)                # [NT, 2]  uniform across cores

    # batches: consecutive tiles, greedily packed so no gather exceeds
    # CAP chunks per table (the SWDGE descriptor ring must hold one whole
    # prepared gather: CAP*128 idxs -> CAP*8 descs per engine).
    CAP = 21
    tiles_of_batch = []
    cur, ca, cb = [], 0, 0
    for t in reversed(range(NT - 4)):
        ta, tb = int(CH[t, 0]), int(CH[t, 1])
        if cur and (ca + ta > CAP or cb + tb > CAP):
            tiles_of_batch.append(cur)
            cur, ca, cb = [], 0, 0
        cur.append(t)
        ca += ta
        cb += tb
    if cur:
        tiles_of_batch.append(cur)
    tiles_of_batch = [sorted(bb) for bb in tiles_of_batch]
    # last two batches: tile pairs of the lowest-degree tiles, so the
    # final drains are tiny and the post-drain compute tail is short
    tiles_of_batch += [[NT - 4, NT - 3], [NT - 2, NT - 1]]

    # chunk-column layout: per batch, chunks grouped PER TILE (A then B)
    # so each tile's selector build is one contiguous is_eq op.
    # gather-index layout: per batch, [A chunks tile-major][B chunks
    # tile-major] (matches the two dma_gather calls).
    NBv = len(tiles_of_batch)
    batch_meta = []        # per batch: dict(nA, nB, icol_A, icol_B)
    tile_meta = {}         # per tile: dict(col0, nA, nB, jA, jB)
    col = 0
    icol = 0
    for b in range(NBv):
        nA = int(sum(CH[t, 0] for t in tiles_of_batch[b]))
        nB = int(sum(CH[t, 1] for t in tiles_of_batch[b]))
        batch_meta.append(dict(nA=nA, nB=nB,
                               icol_A=icol, icol_B=icol + nA * 8))
        jA = 0
        jB = 0
        for t in tiles_of_batch[b]:
            tile_meta[t] = dict(col0=col, nA=int(CH[t, 0]), nB=int(CH[t, 1]),
                                jA=jA, jB=jB)
            col += int(CH[t, 0]) + int(CH[t, 1])
            jA += int(CH[t, 0])
            jB += int(CH[t, 1])
        icol += (nA + nB) * 8
    NCH = col
    NIDX = NCH * 128

    # per-core data arrays
    xT = np.zeros((P, 128, TPAD), bf16)
    disc = np.zeros((P, 128, NT), f32)
    qb = np.zeros((P, TPAD, GP), f32)
    dloc_all = np.full((P, 128, NCH), -1.0, bf16)
    idx_flat = np.zeros((P, NIDX), np.int16)

    for k in range(P):
        ok = order[k * NV:(k + 1) * NV]
        xT[k, :, :NV] = (x[ok] * dis[ok, None]).T.astype(bf16)
        d = np.zeros(TPAD, f32)
        d[:NV] = dis[ok].astype(f32)
        disc[k] = d.reshape(NT, 128).T

    # fill chunk idx / dloc tables (idx layout: per batch, A tile-major
    # then B tile-major; dloc layout: per batch, per tile A then B)
    for b in range(NBv):
        m = batch_meta[b]
        for h, base_icol in ((0, m["icol_A"]), (1, m["icol_B"])):
            jh = 0
            for t in tiles_of_batch[b]:
                nchunk = int(CH[t, h])
                if nchunk > 0:
                    tm = tile_meta[t]
                    for k in range(P):
                        gi = (k * NT + t) * 2 + h
                        g0, g1 = bounds[gi], bounds[gi + 1]
                        n = g1 - g0
                        fbase = base_icol * 16 + jh * 128
                        idx_flat[k, fbase:fbase + n] = idx_s[g0:g1]
                        pp = np.arange(n) % 128
                        cc = np.arange(n) // 128
                        colbase = tm["col0"] + (0 if h == 0 else tm["nA"])
                        dloc_all[k, pp, colbase + cc] = \
                            dloc_s[g0:g1].astype(bf16)
                jh += nchunk
    assert idx_flat.min() >= 0
    assert int(idx_flat.reshape(-1).max()) < HALF
    # wrap gather indices: i -> [i % 16, i // 16], replicated to 128 partitions
    idxs = np.tile(
        idx_flat.reshape(P, NIDX // 16, 16).transpose(0, 2, 1), (1, 8, 1)
    ).astype(np.int16)

    # ---- layer-2 Q blocks: qb[core, pos[src], g] += norm/cnt[g]
    # (self-loops included here)
    e_src = np.concatenate([src, np.arange(N)])
    e_dst = np.concatenate([dst, np.arange(N)])
    g_of = bat[e_dst]
    val = (dis[e_src] * dis[e_dst] / cnt[g_of]).astype(f32)
    np.add.at(qb, (e_src // NV, pos[e_src], g_of), val)
    qb = qb.astype(bf16)

    iota_bf = np.broadcast_to(
        np.arange(128, dtype=bf16), (128, 128)).copy()
    eye16 = np.eye(16, dtype=f32)
    eye128 = np.eye(128, dtype=bf16)
    widx = np.zeros((128, 8), np.int16)

    W1 = np.ascontiguousarray(np.asarray(W1, f32).astype(bf16))
    W2 = np.ascontiguousarray(np.asarray(W2, f32).astype(bf16))
    b1 = np.asarray(b1, f32)
    b2 = np.asarray(b2, f32)
    use_b1 = bool(np.any(b1))
    use_b2 = bool(np.any(b2))

    in_maps = []
    for k in range(P):
        m = {
            "xT": np.ascontiguousarray(xT[k]),
            "qb": np.ascontiguousarray(qb[k]),
            "idxs": np.ascontiguousarray(idxs[k]),
            "dloc": np.ascontiguousarray(dloc_all[k]),
            "disc": np.ascontiguousarray(disc[k]),
            "eye128": eye128,
            "w1": W1, "w2": W2,
            "iota": iota_bf, "eye16": eye16, "widx": widx,
        }
        if use_b1:
            rr = np.zeros((1, TPAD), f32)
            rr[0, :NV] = np.sqrt(deg[order[k * NV:(k + 1) * NV]]).astype(f32)
            m["rdis"] = rr
            m["b1r"] = b1.reshape(1, F)
        if use_b2:
            m["b2r"] = b2.reshape(1, C)
        in_maps.append(m)

    plan = dict(NCH=NCH, NIDX=NIDX, CH=CH, NB=NBv,
                tiles_of_batch=tiles_of_batch,
                batch_meta=batch_meta, tile_meta=tile_meta,
                use_b1=use_b1, use_b2=use_b2)
    return plan, in_maps


# ---------------------------------------------------------------- bass build
def _build(plan):
    dt = mybir.dt
    f32, bf16, i16 = dt.float32, dt.bfloat16, dt.int16
    NCH, NIDX = plan["NCH"], plan["NIDX"]
    use_b1, use_b2 = plan["use_b1"], plan["use_b2"]
    single_packet = bool(int(os.environ.get("GCN_SP", "0")))

    stage = int(os.environ.get("GCN_STAGE", "3"))  # 1: no phase C; 2: +gathers
    nc = bacc.Bacc("TRN2", target_bir_lowering=False, debug=False,
                   num_devices=P, num_swdge_queues=NQ,
                   dynamic_dma_scratch_size=int(os.environ.get(
                       "GCN_SCRATCH", "32768")))
    xT_d = nc.dram_tensor("xT", [128, TPAD], bf16, kind="ExternalInput")
    qb_d = nc.dram_tensor("qb", [TPAD, GP], bf16, kind="ExternalInput")
    idxs_d = nc.dram_tensor("idxs", [128, NIDX // 16], i16, kind="ExternalInput")
    dloc_d = nc.dram_tensor("dloc", [128, NCH], bf16, kind="ExternalInput")
    disc_d = nc.dram_tensor("disc", [128, NT], f32, kind="ExternalInput")
    eye128_d = nc.dram_tensor("eye128", [128, 128], bf16, kind="ExternalInput")
    w1_d = nc.dram_tensor("w1", [F, HID], bf16, kind="ExternalInput")
    w2_d = nc.dram_tensor("w2", [HID, C], bf16, kind="ExternalInput")
    iota_d = nc.dram_tensor("iota", [128, 128], bf16, kind="ExternalInput")
    eye_d = nc.dram_tensor("eye16", [16, 16], f32, kind="ExternalInput")
    widx_d = nc.dram_tensor("widx", [128, 8], i16, kind="ExternalInput")
    if use_b1:
        rdis_d = nc.dram_tensor("rdis", [1, TPAD], f32, kind="ExternalInput")
        b1_d = nc.dram_tensor("b1r", [1, F], f32, kind="ExternalInput")
    if use_b2:
        b2_d = nc.dram_tensor("b2r", [1, C], f32, kind="ExternalInput")
    y_d = nc.dram_tensor("y", [G, C], f32, kind="ExternalOutput")

    with tile.TileContext(nc) as tc:
        cpool = tc.alloc_tile_pool(name="const", bufs=1)
        dram = tc.alloc_tile_pool(name="dram", bufs=1, space="DRAM")

        h_own = dram.tile([NV, F], bf16)
        h_full = dram.tile([N, F], bf16, addr_space="Shared")
        ar_in = dram.tile([128, GP], bf16)
        ar_out = dram.tile([128, GP], bf16)

        # warm gather: preloads the Q7 ext-isa library (~9us) before it's
        # needed; reads a fixed xT row, result unused.
        widx_sb = cpool.tile([128, 8], i16)
        nc.sync.dma_start(widx_sb[:], widx_d[:, :])
        warm_sb = cpool.tile([128, 1, 128], bf16)
        if int(os.environ.get("GCN_WARM", "1")):
            nc.gpsimd.dma_gather(
                out_ap=warm_sb[:], in_ap=xT_d[:, 0:128],
                idxs_ap=widx_sb[:, :], num_idxs=128, num_idxs_reg=128,
                elem_size=F, elem_step=TPAD, single_packet=False,
                queue_num=0)

        # phase-B-critical constants first so their DMAs run first
        w1_sb = cpool.tile([F, HID], bf16)
        nc.sync.dma_start(w1_sb[:], w1_d[:, :])
        disc_sb = cpool.tile([128, NT], f32)
        nc.sync.dma_start(disc_sb[:], disc_d[:, :])
        h_loc = cpool.tile([128, TPAD], bf16)   # local dis*h, node-major tiles
        h1_sb = cpool.tile([128, TPAD], bf16)

        # ---------------- phase B: h = dis * (x @ W1), 2 AllGathers
        with (
            tc.tile_pool(name="xw", bufs=1) as xw,
            tc.tile_pool(name="hp", bufs=4, space="PSUM") as hp,
        ):
            # xT loaded in three parallel DMAs (sync/scalar HWDGE + one
            # SWDGE) -- the load gates every phase-B matmul
            X1, X2 = 16 * 128, 33 * 128
            xa_sb = xw.tile([128, X1], bf16)
            xb_sb = xw.tile([128, X2 - X1], bf16)
            xc_sb = xw.tile([128, TPAD - X2], bf16)
            nc.sync.dma_start(xa_sb[:], xT_d[:, 0:X1])
            nc.scalar.dma_start(xb_sb[:], xT_d[:, X1:X2])
            nc.gpsimd.dma_start(xc_sb[:], xT_d[:, X2:TPAD])
            for t in range(NT):
                ps = hp.tile([128, 128], f32)
                if t * 128 < X1:
                    lhs = xa_sb[:, t * 128:(t + 1) * 128]
                elif t * 128 < X2:
                    lhs = xb_sb[:, t * 128 - X1:(t + 1) * 128 - X1]
                else:
                    lhs = xc_sb[:, t * 128 - X2:(t + 1) * 128 - X2]
                nc.tensor.matmul(ps[:], lhsT=lhs,
                                 rhs=w1_sb[:], start=True, stop=True)
                if t % 2 == 0:
                    nc.vector.tensor_copy(h_loc[:, t * 128:(t + 1) * 128],
                                          ps[:])
                else:
                    nc.scalar.activation(h_loc[:, t * 128:(t + 1) * 128],
                                         ps[:], AF.Copy)
                if t == 24:
                    # tiles 0-24 in ONE transposed-AP DMA: (p, t, f) on
                    # both sides
                    nc.sync.dma_start(
                        h_own[0:3200, :].rearrange("(tt p) f -> p tt f",
                                                   p=128),
                        h_loc[:, 0:3200].rearrange("p (tt f) -> p tt f",
                                                   f=128))
                if t == NT - 1:
                    # tiles 25-47 (23 full tiles), then partial tile 48
                    nc.scalar.dma_start(
                        h_own[3200:6144, :].rearrange("(tt p) f -> p tt f",
                                                      p=128),
                        h_loc[:, 3200:6144].rearrange("p (tt f) -> p tt f",
                                                      f=128))
                    nc.scalar.dma_start(
                        h_own[6144:NV, :],
                        h_loc[0:NV - 6144, 6144:6272])
            nc.gpsimd.collective_compute(
                "AllGather", ALU.bypass, replica_groups=[list(range(P))],
                ins=[h_own[:].opt()], outs=[h_full[:].opt()])

        # remaining phase-C constants (issued after phase B so their DMAs
        # don't delay the h_own writes on the HWDGE queues)
        idxs_sb = cpool.tile([128, NIDX // 16], i16)
        nc.scalar.dma_start(idxs_sb[:], idxs_d[:, :])
        dloc_sb = cpool.tile([128, NCH], bf16)
        nc.scalar.dma_start(dloc_sb[:], dloc_d[:, :])
        iota_sb = cpool.tile([128, 128], bf16)
        nc.sync.dma_start(iota_sb[:], iota_d[:, :])
        eye_sb = cpool.tile([16, 16], f32)
        nc.sync.dma_start(eye_sb[:], eye_d[:, :])
        eye128_sb = cpool.tile([128, 128], bf16)
        nc.sync.dma_start(eye128_sb[:], eye128_d[:, :])
        w2_sb = cpool.tile([HID, C], bf16)
        nc.sync.dma_start(w2_sb[:], w2_d[:, :])
        if use_b1:
            rdis_sb = cpool.tile([1, TPAD], f32)
            nc.sync.dma_start(rdis_sb[:], rdis_d[:, :])
            b1_sb = cpool.tile([1, F], f32)
            nc.sync.dma_start(b1_sb[:], b1_d[:, :])
        if use_b2:
            b2_sb = cpool.tile([1, C], f32)
            nc.sync.dma_start(b2_sb[:], b2_d[:, :])

        # ---------------- phase C: layer-1 aggregation + layer-2 contraction
        with tc.tile_pool(name="ptp", bufs=1, space="PSUM") as ptp:
            poolT = ptp.tile([128, GP], f32)
            i_l2 = 0
            with (
                tc.tile_pool(name="ga", bufs=6) as ga_p,
                tc.tile_pool(name="gb", bufs=6) as gb_p,
                tc.tile_pool(name="selp", bufs=8) as selp,
                tc.tile_pool(name="qp", bufs=4) as qp,
                tc.tile_pool(name="aggp", bufs=7, space="PSUM") as aggp,
            ):
                l2q = []
                NBv = plan["NB"]
                ngb = int(os.environ.get("GCN_NGB", str(NBv)))
                # PREPARE_ONLY descriptor generation: the prep only reads
                # idxs (gen runs during phase B / AllGather); the h_full
                # RAW dep moves to the trigger. Ring FIFO order per queue
                # keeps batch completion in issue order. Lead with one
                # prep per queue, then strict prep->trigger pairs so a
                # blocked prep (pool WAR) never delays an earlier trigger.
                gAs = {}
                gBs = {}
                issue = []
                for b in range(ngb):
                    issue.append(("A", b))
                    issue.append(("B", b))
                gi = 0
                for side, b in issue:
                    if stage < 2:
                        continue
                    m = plan["batch_meta"][b]
                    nA, nB = m["nA"], m["nB"]
                    q = gi % NQ
                    if side == "A" and nA:
                        gA = ga_p.tile([128, nA, 128], bf16, tag="ga")
                        gAs[b] = gA
                        nc.gpsimd.dma_gather(
                            out_ap=gA[:], in_ap=h_full[0:HALF, :],
                            idxs_ap=idxs_sb[:, m["icol_A"]:
                                            m["icol_A"] + nA * 8],
                            num_idxs=nA * 128, num_idxs_reg=nA * 128,
                            elem_size=F, single_packet=single_packet,
                            queue_num=q)
                    elif side == "B" and nB:
                        gB = gb_p.tile([128, nB, 128], bf16, tag="gb")
                        gBs[b] = gB
                        nc.gpsimd.dma_gather(
                            out_ap=gB[:], in_ap=h_full[HALF:N, :],
                            idxs_ap=idxs_sb[:, m["icol_B"]:
                                            m["icol_B"] + nB * 8],
                            num_idxs=nB * 128, num_idxs_reg=nB * 128,
                            elem_size=F, single_packet=single_packet,
                            queue_num=q)
                    else:
                        continue
                    gi += 1
                for b in range(NBv):
                    if b >= ngb:
                        continue
                    gA = gAs.get(b)
                    gB = gBs.get(b)
                    if stage < 3:
                        if stage == 2 and (gA is not None or gB is not None):
                            junk = selp.tile([128, 128], bf16, tag="sel")
                            gj = gA if gA is not None else gB
                            nc.vector.tensor_copy(junk[:], gj[:, 0, :])
                            nc.sync.dma_start(ar_in[0:128, 0:64],
                                              junk[:, 0:64])
                        continue
                    for t in plan["tiles_of_batch"][b]:
                        tm = plan["tile_meta"][t]
                        t_nA, t_nB = tm["nA"], tm["nB"]
                        nch_t = t_nA + t_nB
                        # one is_eq builds all selectors for this tile
                        sel = selp.tile([128, nch_t, 128], bf16, tag="sel")
                        nc.vector.tensor_tensor(
                            out=sel[:],
                            in0=iota_sb[:, None, :].to_broadcast(
                                [128, nch_t, 128]),
                            in1=dloc_sb[:, tm["col0"]:tm["col0"] + nch_t,
                                        None].to_broadcast([128, nch_t, 128]),
                            op=ALU.is_equal)
                        ps = aggp.tile([128, 128], f32, tag="agg")
                        # self-loop: the message is h_loc itself; identity
                        # lhsT adds it into the PSUM accumulation
                        nc.tensor.matmul(
                            ps[:], lhsT=eye128_sb[:],
                            rhs=h_loc[:, t * 128:(t + 1) * 128],
                            start=True, stop=False)
                        if use_b1:
                            nc.tensor.matmul(
                                ps[:], lhsT=rdis_sb[0:1, t * 128:(t + 1) * 128],
                                rhs=b1_sb[:], start=False, stop=False)
                        for ci in range(nch_t):
                            if ci < t_nA:
                                gsrc, joff = gA, tm["jA"] + ci
                            else:
                                gsrc, joff = gB, tm["jB"] + (ci - t_nA)
                            nc.tensor.matmul(
                                ps[:], lhsT=sel[:, ci, :],
                                rhs=gsrc[:, joff, :],
                                start=False, stop=(ci == nch_t - 1))
                        nc.scalar.activation(
                            h1_sb[:, t * 128:(t + 1) * 128], ps[:], AF.Relu,
                            scale=disc_sb[:, t:t + 1])
                        # layer 2: poolT += H1_tile^T-contraction with Q
                        # block, deferred by one tile so the PE stream does
                        # not stall waiting for this tile's relu.
                        qt = qp.tile([128, GP], bf16, tag="q")
                        nc.sync.dma_start(
                            qt[:], qb_d[t * 128:(t + 1) * 128, :])
                        l2q.append((t, qt))
                        if len(l2q) > 1:
                            tp_, qtp = l2q.pop(0)
                            nc.tensor.matmul(
                                poolT[:],
                                lhsT=h1_sb[:, tp_ * 128:(tp_ + 1) * 128],
                                rhs=qtp[:],
                                start=(i_l2 == 0), stop=False)
                            i_l2 += 1

                while l2q:
                    tp_, qtp = l2q.pop(0)
                    nc.tensor.matmul(
                        poolT[:],
                        lhsT=h1_sb[:, tp_ * 128:(tp_ + 1) * 128],
                        rhs=qtp[:],
                        start=(i_l2 == 0), stop=(i_l2 == NT - 1))
                    i_l2 += 1

            pt_sb = cpool.tile([128, GP], bf16)
            if stage >= 3:
                nc.scalar.activation(pt_sb[:], poolT[:], AF.Copy)
            else:
                nc.vector.memset(pt_sb[:], 0.0)
            nc.sync.dma_start(ar_in[:], pt_sb[:])

        nc.gpsimd.collective_compute(
            "AllReduce", ALU.add, replica_groups=[list(range(P))],
            ins=[ar_in[:].opt()], outs=[ar_out[:].opt()])

        # ---------------- phase D: W2, bias, log_softmax
        with (
            tc.tile_pool(name="fin", bufs=1) as fin,
            tc.tile_pool(name="fps", bufs=2, space="PSUM") as fps,
            tc.tile_pool(name="sm", bufs=4) as smp,
        ):
            pooledT = fin.tile([128, GP], bf16)
            nc.sync.dma_start(pooledT[:], ar_out[:])
            out2 = fps.tile([16, GP], f32, tag="out2")
            nc.tensor.matmul(out2[:], lhsT=w2_sb[:], rhs=pooledT[:],
                             start=True, stop=not use_b2)
            if use_b2:
                ones = fin.tile([1, GP], f32)
                nc.vector.memset(ones[:], 1.0)
                nc.tensor.matmul(out2[:], lhsT=b2_sb[:], rhs=ones[:],
                                 start=False, stop=True)
            logitsT = fin.tile([16, GP], f32)
            nc.scalar.activation(logitsT[:], out2[:], AF.Copy)
            for gt in range(min(GT, -(-G // 128))):
                tp = fps.tile([128, 16], f32, tag="tp")
                nc.tensor.transpose(
                    tp[:], logitsT[:, gt * 128:(gt + 1) * 128], eye_sb[:])
                nmx = smp.tile([128, 1], f32, tag="nmx")
                nc.vector.reduce_max(out=nmx[:], in_=tp[:],
                                     axis=mybir.AxisListType.X, negate=True)
                ex = smp.tile([128, 16], f32, tag="ex")
                nc.scalar.activation(ex[:], tp[:], AF.Exp, bias=nmx[:, 0:1])
                sm = smp.tile([128, 1], f32, tag="sm")
                nc.vector.reduce_sum(out=sm[:], in_=ex[:],
                                     axis=mybir.AxisListType.X)
                lse = smp.tile([128, 1], f32, tag="lse")
                nc.scalar.activation(lse[:], sm[:], AF.Ln)
                res = smp.tile([128, 16], f32, tag="res")
                nc.vector.tensor_scalar(res[:], tp[:], nmx[:, 0:1],
                                        lse[:, 0:1], ALU.add, ALU.subtract)
                rows = min(128, G - gt * 128)
                nc.sync.dma_start(y_d[gt * 128:gt * 128 + rows, :],
                                  res[0:rows, :])
        dram.release()
        cpool.release()
    nc.compile()
    return nc


# ---------------------------------------------------------------- entry
def kernel(x, W1, b1, W2, b2, edge_src, edge_dst, batch):
    global LAST_EXEC_NS, LAST_RESULT
    plan, in_maps = _preprocess(x, W1, b1, W2, b2,
                                edge_src, edge_dst, batch)
    nc = _build(plan)
    trace = bool(int(os.environ.get("GCN_TRACE", "0")))
    kw = {}
    if trace and _install_profile_hook():
        kw = dict(trace=True, trace_cores=[0])
    res = run_bass_kernel_spmd(nc, in_maps, core_ids=list(range(P)), **kw)
    LAST_RESULT = res
    LAST_EXEC_NS = res.exec_time_ns
    return np.ascontiguousarray(res.results[0]["y"].astype(np.float32))
